# revision 39
# baseline (speedup 1.0000x reference)
"""Bahdanau additive attention kernel for Trainium2 (8 NeuronCores, SPMD).

Problem (per reference):
    ctx_  = ctx @ Wc.T          [S,B,MID]   (dominant cost: S*B*C*MID MACs)
    hid_  = hid @ Wh.T          [1,B,MID]
    scores= tanh(ctx_+hid_) @ wm[0]  -> [S,B]
    alpha = softmax_S(mask(scores))
    z     = sum_s alpha * ctx   [B,C]
    z_t   = z @ Wo.T            [B,H]
    returns (alpha, z_t)

Sharding: data-parallel over B (64) across 8 cores -> 8 batches/core.
Device layout (per core); matmul tensors are float32r (fp32 bytes, PE
runs them at bf16 rate for moving dim >= 256 with near-fp32 accuracy):
    xt      [C=1024, b=8, S=1024]   ctx shard transposed (c on partitions)
    hidp    [k=8, p=128, b=8]       hid shard transposed + tiled over H
    wmp     [p=128, j=8]            wm tiled over MID
    maskadd [b=8, S=1024]           0 where mask>0 else -1e8 (additive)
    wct/wht/wot [1024,1024]         Wc.T / Wh.T / Wo.T (replicated)
Outputs: alpha_o [b=8, S=1024], zt_o [b=8, H=1024].

Main loop per batch b: for each 128-wide MID tile j, accumulate
P.T[j] = Wc.T[:,j].T @ xt[:, b, :] over 8 c-tiles (PE, float32r, N=512),
tanh(+hid bias) on ACT into SBUF, then PE-accumulate wm.T @ tanh -> scores.
Softmax on one partition (no max-sub needed: |scores| <= sum|wm| ~ 25).
alpha broadcast to 128 partitions (gpsimd), then DVE mul + reduce_sum
computes zT[c,b] = sum_s xt[c,s]*alpha[s] per c-tile.
Final z_t = zT.T @ Wo.T on PE.
"""

import sys

import numpy as np

for _p in ("/opt/trn_rl_repo", "/root/.axon_site/_ro/trn_rl_repo"):
    if _p not in sys.path:
        sys.path.insert(0, _p)

S = 1024
C = 1024
H = 1024
MID = 1024
BL = 8  # batches per core
NCORES = 8
KT = 8  # 128-wide tiles over C/H/MID

TRACE = False
TRACE_KWARGS = {}

_CACHED = {}


def _build():
    import concourse.tile as tile
    from concourse import bacc, mybir

    f32 = mybir.dt.float32
    f32r = mybir.dt.float32r
    f16 = mybir.dt.float16
    AF = mybir.ActivationFunctionType
    AX = mybir.AxisListType

    nc = bacc.Bacc("TRN2", target_bir_lowering=False, debug=False,
                   num_devices=NCORES)

    xt = nc.dram_tensor("xt", [C, BL, S], f32r, kind="ExternalInput").ap()
    hidp = nc.dram_tensor("hidp", [128, KT * BL], f16, kind="ExternalInput").ap()
    wmp = nc.dram_tensor("wmp", [128, KT], f32r, kind="ExternalInput").ap()
    maskadd = nc.dram_tensor("maskadd", [BL, S], f32, kind="ExternalInput").ap()
    wct = nc.dram_tensor("wct", [C, MID], f32r, kind="ExternalInput").ap()
    wht = nc.dram_tensor("wht", [H, MID], f16, kind="ExternalInput").ap()
    wot = nc.dram_tensor("wot", [C, H], f32r, kind="ExternalInput").ap()
    alpha_o = nc.dram_tensor("alpha_o", [BL, S], f32, kind="ExternalOutput").ap()
    zt_o = nc.dram_tensor("zt_o", [BL, H], f32, kind="ExternalOutput").ap()

    with tile.TileContext(nc) as tc:
        consts = tc.alloc_tile_pool(name="consts", bufs=1)
        # resident weights / small constants
        wct_sb = [consts.tile([128, MID], f32r, name=f"wct{k}") for k in range(KT)]
        wot_sb = [consts.tile([128, H], f32r, name=f"wot{k}") for k in range(KT)]
        hidp_sb = consts.tile([128, KT * BL], f16, name="hidp_sb")
        hid_sb = [consts.tile([128, BL], f32, name=f"hid{j}") for j in range(KT)]
        wm_sb = consts.tile([128, KT], f32r, name="wm_sb")
        mask_sb = consts.tile([1, BL * S], f32, name="mask_sb")
        zT_sb = [consts.tile([128, BL], f32, name=f"zT{k}") for k in range(KT)]

        nc.sync.dma_start(out=hidp_sb, in_=hidp)

        # ---- hid projection: hid_T[m, b] = sum_h WhT[h, m] * hidT[h, b] ----
        whtp = tc.alloc_tile_pool(name="whtp", bufs=2)
        hps = tc.alloc_tile_pool(name="hps", bufs=1, space="PSUM")
        hid_ps = [hps.tile([128, BL], f32, name=f"hidps{j}", tag=f"hidps{j}")
                  for j in range(KT)]
        for k in range(KT):
            whtt = whtp.tile([128, MID], f16, name=f"whtt{k}", tag="wht")
            nc.sync.dma_start(out=whtt, in_=wht[k * 128:(k + 1) * 128, :])
            for j in range(KT):
                nc.tensor.matmul(
                    hid_ps[j], lhsT=whtt[:, j * 128:(j + 1) * 128],
                    rhs=hidp_sb[:, k * BL:(k + 1) * BL],
                    start=(k == 0), stop=(k == KT - 1))
        for j in range(KT):
            nc.vector.tensor_copy(hid_sb[j], hid_ps[j])
        hps.release()
        whtp.release()

        for k in range(KT):
            nc.sync.dma_start(out=wct_sb[k][:, 0:512],
                              in_=wct[k * 128:(k + 1) * 128, 0:512])
        nc.sync.dma_start(out=wm_sb, in_=wmp)
        nc.sync.dma_start(out=mask_sb,
                          in_=maskadd.rearrange("b s -> (b s)").unsqueeze(0))

        # ---- main per-batch loop ----
        xtp = tc.alloc_tile_pool(name="xtp", bufs=16)
        tanp = tc.alloc_tile_pool(name="tanp", bufs=3)
        scrp = tc.alloc_tile_pool(name="scrp", bufs=2)
        abcp = tc.alloc_tile_pool(name="abcp", bufs=2)
        smallp = tc.alloc_tile_pool(name="smallp", bufs=2)
        pp = tc.alloc_tile_pool(name="pp", bufs=2, space="PSUM")
        scp = tc.alloc_tile_pool(name="scp", bufs=4, space="PSUM")

        sume_row = consts.tile([1, BL], f32, name="sume_row")
        rcp_col = consts.tile([BL, 1], f32, name="rcp_col")
        rcp_row = consts.tile([1, BL], f32, name="rcp_row")

        for b in range(BL):
            xtt = [xtp.tile([128, S], f32r, name=f"xt{b}_{k}", tag="xt")
                   for k in range(KT)]
            if b == 0:
                # h=0 halves first so j0's first matmuls can start sooner
                for k in range(KT):
                    nc.sync.dma_start(out=xtt[k][:, 0:512],
                                      in_=xt[k * 128:(k + 1) * 128, b, 0:512])
                for k in range(KT):
                    nc.sync.dma_start(out=xtt[k][:, 512:1024],
                                      in_=xt[k * 128:(k + 1) * 128, b, 512:1024])
                for k in range(KT):
                    nc.sync.dma_start(out=wct_sb[k][:, 512:1024],
                                      in_=wct[k * 128:(k + 1) * 128, 512:1024])
            else:
                for k in range(KT):
                    nc.sync.dma_start(out=xtt[k],
                                      in_=xt[k * 128:(k + 1) * 128, b, :])

            sc_a = scp.tile([1, 512], f32, name=f"sca{b}", tag="sc")
            sc_b = scp.tile([1, 512], f32, name=f"scb{b}", tag="sc")
            for j in range(KT):
                pt = pp.tile([128, S], f32, name=f"pt{b}_{j}", tag="pp")
                for k in range(KT):
                    for h in range(2):
                        nc.tensor.matmul(
                            pt[:, h * 512:(h + 1) * 512],
                            lhsT=wct_sb[k][:, j * 128:(j + 1) * 128],
                            rhs=xtt[k][:, h * 512:(h + 1) * 512],
                            start=(k == 0), stop=(k == KT - 1))
                tt = tanp.tile([128, S], f32r, name=f"tt{b}_{j}", tag="T")
                nc.scalar.activation(out=tt, in_=pt, func=AF.Tanh,
                                     bias=hid_sb[j][:, b:b + 1], scale=1.0)
                nc.tensor.matmul(sc_a, lhsT=wm_sb[:, j:j + 1],
                                 rhs=tt[:, 0:512],
                                 start=(j == 0), stop=(j == KT - 1))
                nc.tensor.matmul(sc_b, lhsT=wm_sb[:, j:j + 1],
                                 rhs=tt[:, 512:1024],
                                 start=(j == 0), stop=(j == KT - 1))

            # scores (+ additive mask) on partition 0
            scores = smallp.tile([1, S], f32, name=f"scores{b}", tag="scores")
            nc.vector.tensor_add(scores[:, 0:512], sc_a,
                                 mask_sb[:, b * S:b * S + 512])
            nc.vector.tensor_add(scores[:, 512:1024], sc_b,
                                 mask_sb[:, b * S + 512:b * S + 1024])

            # softmax over S (no max subtraction; |scores| is bounded ~25).
            # Normalization is deferred: broadcast raw exp, scale z_t at end.
            nc.scalar.activation(out=scores, in_=scores, func=AF.Exp,
                                 accum_out=sume_row[:, b:b + 1])

            # weighted (unnormalized) context: zT[c,b] = sum_s xt[c,s]*e[s]
            abc = abcp.tile([128, S], f32, name=f"abc{b}", tag="abc")
            nc.gpsimd.partition_broadcast(abc, scores)
            for k in range(KT):
                prod = scrp.tile([128, S], f32, name=f"prod{b}_{k}", tag="prod")
                nc.vector.tensor_mul(prod, xtt[k].bitcast(f32), abc)
                nc.scalar.activation(out=prod, in_=prod, func=AF.Copy,
                                     accum_out=zT_sb[k][:, b:b + 1])

            # normalized alpha output (off the critical chain)
            rcp = smallp.tile([1, 1], f32, name=f"rcp{b}", tag="rcp")
            nc.vector.reciprocal(rcp, sume_row[:, b:b + 1])
            nc.vector.tensor_scalar_mul(scores, scores, rcp)
            nc.sync.dma_start(out=alpha_o[b:b + 1, :], in_=scores)

        # ---- z_t = zT.T @ WoT ----
        scp.release()
        pp.release()
        for k in range(KT):
            nc.sync.dma_start(out=wot_sb[k], in_=wot[k * 128:(k + 1) * 128, :])
        zTr = [consts.tile([128, BL], f32r, name=f"zTr{k}") for k in range(KT)]
        ztp = tc.alloc_tile_pool(name="ztp", bufs=1, space="PSUM")
        zt_ps = [ztp.tile([BL, 512], f32, name=f"ztps{h}", tag=f"ztps{h}")
                 for h in range(2)]
        for k in range(KT):
            nc.vector.tensor_copy(zTr[k], zT_sb[k])
            for h in range(2):
                nc.tensor.matmul(zt_ps[h], lhsT=zTr[k],
                                 rhs=wot_sb[k][:, h * 512:(h + 1) * 512],
                                 start=(k == 0), stop=(k == KT - 1))
        nc.vector.reciprocal(rcp_row, sume_row)
        nc.sync.dma_start(out=rcp_col, in_=rcp_row)
        zt_sb = smallp.tile([BL, H], f32, name="zt_sb", tag="ztsb")
        for h in range(2):
            nc.vector.tensor_scalar_mul(zt_sb[:, h * 512:(h + 1) * 512],
                                        zt_ps[h], rcp_col)
        nc.sync.dma_start(out=zt_o, in_=zt_sb)

        # release in LIFO order per space (SBUF / PSUM stacks)
        ztp.release()
        smallp.release()
        abcp.release()
        scrp.release()
        tanp.release()
        xtp.release()
        consts.release()

    nc.compile()
    return nc


def _get_nc():
    if "nc" not in _CACHED:
        _CACHED["nc"] = _build()
    return _CACHED["nc"]


def kernel(hid, ctx, ctx_mask, Wc, Wh, wm, Wo):
    from concourse.bass_utils import run_bass_kernel_spmd

    hid = np.asarray(hid, dtype=np.float32)
    ctx = np.asarray(ctx, dtype=np.float32)
    ctx_mask = np.asarray(ctx_mask, dtype=np.float32)
    Wc = np.asarray(Wc, dtype=np.float32)
    Wh = np.asarray(Wh, dtype=np.float32)
    wm = np.asarray(wm, dtype=np.float32)
    Wo = np.asarray(Wo, dtype=np.float32)

    B = ctx.shape[1]
    assert ctx.shape == (S, B, C) and B == NCORES * BL

    import ml_dtypes
    wct = np.ascontiguousarray(Wc.T)
    wht = np.ascontiguousarray(Wh.T).astype(np.float16)
    wot = np.ascontiguousarray(Wo.T)
    wmp = np.ascontiguousarray(wm[0].reshape(KT, 128).T)

    in_maps = []
    for core in range(NCORES):
        b0 = core * BL
        xt = np.ascontiguousarray(ctx[:, b0:b0 + BL, :].transpose(2, 1, 0))
        hidp = np.ascontiguousarray(
            hid[0, b0:b0 + BL, :].T.reshape(KT, 128, BL)
            .transpose(1, 0, 2).reshape(128, KT * BL)).astype(np.float16)
        madd = np.ascontiguousarray(
            np.where(ctx_mask[:, b0:b0 + BL] > 0, 0.0, -1e8).T
        ).astype(np.float32)
        in_maps.append(dict(xt=xt, hidp=hidp, wmp=wmp, maskadd=madd,
                            wct=wct, wht=wht, wot=wot))

    nc = _get_nc()
    res = run_bass_kernel_spmd(nc, in_maps, list(range(NCORES)),
                               trace=TRACE, **TRACE_KWARGS)
    _CACHED["last_result"] = res

    alpha = np.empty((S, B), dtype=np.float32)
    zt = np.empty((B, H), dtype=np.float32)
    for core in range(NCORES):
        b0 = core * BL
        alpha[:, b0:b0 + BL] = res.results[core]["alpha_o"].T
        zt[b0:b0 + BL, :] = res.results[core]["zt_o"]
    return alpha, zt


# revision 41
# speedup vs baseline: 1.0088x; 1.0088x over previous
"""Bahdanau additive attention kernel for Trainium2 (8 NeuronCores, SPMD).

Problem (per reference):
    ctx_  = ctx @ Wc.T          [S,B,MID]   (dominant cost: S*B*C*MID MACs)
    hid_  = hid @ Wh.T          [1,B,MID]
    scores= tanh(ctx_+hid_) @ wm[0]  -> [S,B]
    alpha = softmax_S(mask(scores))
    z     = sum_s alpha * ctx   [B,C]
    z_t   = z @ Wo.T            [B,H]
    returns (alpha, z_t)

Sharding: data-parallel over B (64) across 8 cores -> 8 batches/core.
Device layout (per core); matmul tensors are float32r (fp32 bytes, PE
runs them at bf16 rate for moving dim >= 256 with near-fp32 accuracy):
    xt      [C=1024, b=8, S=1024]   ctx shard transposed (c on partitions)
    hidp    [k=8, p=128, b=8]       hid shard transposed + tiled over H
    wmp     [p=128, j=8]            wm tiled over MID
    maskadd [b=8, S=1024]           0 where mask>0 else -1e8 (additive)
    wct/wht/wot [1024,1024]         Wc.T / Wh.T / Wo.T (replicated)
Outputs: alpha_o [b=8, S=1024], zt_o [b=8, H=1024].

Main loop per batch b: for each 128-wide MID tile j, accumulate
P.T[j] = Wc.T[:,j].T @ xt[:, b, :] over 8 c-tiles (PE, float32r, N=512),
tanh(+hid bias) on ACT into SBUF, then PE-accumulate wm.T @ tanh -> scores.
Softmax on one partition (no max-sub needed: |scores| <= sum|wm| ~ 25).
alpha broadcast to 128 partitions (gpsimd), then DVE mul + reduce_sum
computes zT[c,b] = sum_s xt[c,s]*alpha[s] per c-tile.
Final z_t = zT.T @ Wo.T on PE.
"""

import sys

import numpy as np

for _p in ("/opt/trn_rl_repo", "/root/.axon_site/_ro/trn_rl_repo"):
    if _p not in sys.path:
        sys.path.insert(0, _p)

S = 1024
C = 1024
H = 1024
MID = 1024
BL = 8  # batches per core
NCORES = 8
KT = 8  # 128-wide tiles over C/H/MID

TRACE = False
TRACE_KWARGS = {}

_CACHED = {}


def _build():
    import concourse.tile as tile
    from concourse import bacc, mybir

    f32 = mybir.dt.float32
    f32r = mybir.dt.float32r
    f16 = mybir.dt.float16
    AF = mybir.ActivationFunctionType
    AX = mybir.AxisListType

    nc = bacc.Bacc("TRN2", target_bir_lowering=False, debug=False,
                   num_devices=NCORES)

    xt = nc.dram_tensor("xt", [C, BL, S], f32r, kind="ExternalInput").ap()
    hidp = nc.dram_tensor("hidp", [128, KT * BL], f16, kind="ExternalInput").ap()
    wmp = nc.dram_tensor("wmp", [128, KT], f32r, kind="ExternalInput").ap()
    maskadd = nc.dram_tensor("maskadd", [BL, S], f32, kind="ExternalInput").ap()
    wct = nc.dram_tensor("wct", [C, MID], f32r, kind="ExternalInput").ap()
    wht = nc.dram_tensor("wht", [H, MID], f16, kind="ExternalInput").ap()
    wot = nc.dram_tensor("wot", [C, H], f32r, kind="ExternalInput").ap()
    alpha_o = nc.dram_tensor("alpha_o", [BL, S], f32, kind="ExternalOutput").ap()
    zt_o = nc.dram_tensor("zt_o", [BL, H], f32, kind="ExternalOutput").ap()

    with tile.TileContext(nc) as tc:
        consts = tc.alloc_tile_pool(name="consts", bufs=1)
        # resident weights / small constants
        wct_sb = [consts.tile([128, MID], f32r, name=f"wct{k}") for k in range(KT)]
        wot_sb = [consts.tile([128, H], f32r, name=f"wot{k}") for k in range(KT)]
        hidp_sb = consts.tile([128, KT * BL], f16, name="hidp_sb")
        hid_sb = [consts.tile([128, BL], f32, name=f"hid{j}") for j in range(KT)]
        wm_sb = consts.tile([128, KT], f32r, name="wm_sb")
        mask_sb = consts.tile([1, BL * S], f32, name="mask_sb")
        zT_sb = [consts.tile([128, BL], f32, name=f"zT{k}") for k in range(KT)]

        nc.sync.dma_start(out=hidp_sb, in_=hidp)

        # ---- hid projection: hid_T[m, b] = sum_h WhT[h, m] * hidT[h, b] ----
        whtp = tc.alloc_tile_pool(name="whtp", bufs=2)
        hps = tc.alloc_tile_pool(name="hps", bufs=1, space="PSUM")
        hid_ps = [hps.tile([128, BL], f32, name=f"hidps{j}", tag=f"hidps{j}")
                  for j in range(KT)]
        for k in range(KT):
            whtt = whtp.tile([128, MID], f16, name=f"whtt{k}", tag="wht")
            nc.sync.dma_start(out=whtt, in_=wht[k * 128:(k + 1) * 128, :])
            for j in range(KT):
                nc.tensor.matmul(
                    hid_ps[j], lhsT=whtt[:, j * 128:(j + 1) * 128],
                    rhs=hidp_sb[:, k * BL:(k + 1) * BL],
                    start=(k == 0), stop=(k == KT - 1))
        for j in range(KT):
            nc.vector.tensor_copy(hid_sb[j], hid_ps[j])
        hps.release()
        whtp.release()

        for k in range(KT):
            nc.sync.dma_start(out=wct_sb[k][:, 0:512],
                              in_=wct[k * 128:(k + 1) * 128, 0:512])
        nc.sync.dma_start(out=wm_sb, in_=wmp)
        nc.sync.dma_start(out=mask_sb,
                          in_=maskadd.rearrange("b s -> (b s)").unsqueeze(0))

        # ---- main per-batch loop ----
        xtp = tc.alloc_tile_pool(name="xtp", bufs=16)
        tanp = tc.alloc_tile_pool(name="tanp", bufs=3)
        scrp = tc.alloc_tile_pool(name="scrp", bufs=2)
        abcp = tc.alloc_tile_pool(name="abcp", bufs=2)
        smallp = tc.alloc_tile_pool(name="smallp", bufs=2)
        pp = tc.alloc_tile_pool(name="pp", bufs=2, space="PSUM")
        scp = tc.alloc_tile_pool(name="scp", bufs=4, space="PSUM")

        sume_row = consts.tile([1, BL], f32, name="sume_row")
        rcp_col = consts.tile([BL, 1], f32, name="rcp_col")
        rcp_row = consts.tile([1, BL], f32, name="rcp_row")

        for b in range(BL):
            xtt = [xtp.tile([128, S], f32r, name=f"xt{b}_{k}", tag="xt")
                   for k in range(KT)]
            for k in range(KT):
                nc.sync.dma_start(out=xtt[k],
                                  in_=xt[k * 128:(k + 1) * 128, b, :])
            if b == 0:
                for k in range(KT):
                    nc.sync.dma_start(out=wct_sb[k][:, 512:1024],
                                      in_=wct[k * 128:(k + 1) * 128, 512:1024])

            sc_a = scp.tile([1, 512], f32, name=f"sca{b}", tag="sc")
            sc_b = scp.tile([1, 512], f32, name=f"scb{b}", tag="sc")
            for j in range(KT):
                pt = pp.tile([128, S], f32, name=f"pt{b}_{j}", tag="pp")
                for k in range(KT):
                    for h in range(2):
                        nc.tensor.matmul(
                            pt[:, h * 512:(h + 1) * 512],
                            lhsT=wct_sb[k][:, j * 128:(j + 1) * 128],
                            rhs=xtt[k][:, h * 512:(h + 1) * 512],
                            start=(k == 0), stop=(k == KT - 1))
                tt = tanp.tile([128, S], f32r, name=f"tt{b}_{j}", tag="T")
                nc.scalar.activation(out=tt, in_=pt, func=AF.Tanh,
                                     bias=hid_sb[j][:, b:b + 1], scale=1.0)
                nc.tensor.matmul(sc_a, lhsT=wm_sb[:, j:j + 1],
                                 rhs=tt[:, 0:512],
                                 start=(j == 0), stop=(j == KT - 1))
                nc.tensor.matmul(sc_b, lhsT=wm_sb[:, j:j + 1],
                                 rhs=tt[:, 512:1024],
                                 start=(j == 0), stop=(j == KT - 1))

            # scores (+ additive mask) on partition 0
            scores = smallp.tile([1, S], f32, name=f"scores{b}", tag="scores")
            nc.vector.tensor_add(scores[:, 0:512], sc_a,
                                 mask_sb[:, b * S:b * S + 512])
            nc.vector.tensor_add(scores[:, 512:1024], sc_b,
                                 mask_sb[:, b * S + 512:b * S + 1024])

            # softmax over S (no max subtraction; |scores| is bounded ~25).
            # Normalization is deferred: broadcast raw exp, scale z_t at end.
            nc.scalar.activation(out=scores, in_=scores, func=AF.Exp,
                                 accum_out=sume_row[:, b:b + 1])

            # weighted (unnormalized) context: zT[c,b] = sum_s xt[c,s]*e[s]
            abc = abcp.tile([128, S], f32, name=f"abc{b}", tag="abc")
            nc.gpsimd.partition_broadcast(abc, scores)
            for k in range(KT):
                prod = scrp.tile([128, S], f32, name=f"prod{b}_{k}", tag="prod")
                nc.vector.tensor_mul(prod, xtt[k].bitcast(f32), abc)
                nc.scalar.activation(out=prod, in_=prod, func=AF.Copy,
                                     accum_out=zT_sb[k][:, b:b + 1])

            # normalized alpha output (off the critical chain)
            rcp = smallp.tile([1, 1], f32, name=f"rcp{b}", tag="rcp")
            nc.vector.reciprocal(rcp, sume_row[:, b:b + 1])
            nc.vector.tensor_scalar_mul(scores, scores, rcp)
            nc.sync.dma_start(out=alpha_o[b:b + 1, :], in_=scores)

        # ---- z_t = zT.T @ WoT ----
        scp.release()
        pp.release()
        for k in range(KT):
            nc.sync.dma_start(out=wot_sb[k], in_=wot[k * 128:(k + 1) * 128, :])
        zTr = [consts.tile([128, BL], f32r, name=f"zTr{k}") for k in range(KT)]
        ztp = tc.alloc_tile_pool(name="ztp", bufs=1, space="PSUM")
        zt_ps = [ztp.tile([BL, 512], f32, name=f"ztps{h}", tag=f"ztps{h}")
                 for h in range(2)]
        for k in range(KT):
            nc.vector.tensor_copy(zTr[k], zT_sb[k])
            for h in range(2):
                nc.tensor.matmul(zt_ps[h], lhsT=zTr[k],
                                 rhs=wot_sb[k][:, h * 512:(h + 1) * 512],
                                 start=(k == 0), stop=(k == KT - 1))
        nc.vector.reciprocal(rcp_row, sume_row)
        nc.sync.dma_start(out=rcp_col, in_=rcp_row)
        zt_sb = smallp.tile([BL, H], f32, name="zt_sb", tag="ztsb")
        for h in range(2):
            nc.vector.tensor_scalar_mul(zt_sb[:, h * 512:(h + 1) * 512],
                                        zt_ps[h], rcp_col)
        nc.sync.dma_start(out=zt_o, in_=zt_sb)

        # release in LIFO order per space (SBUF / PSUM stacks)
        ztp.release()
        smallp.release()
        abcp.release()
        scrp.release()
        tanp.release()
        xtp.release()
        consts.release()

    nc.compile()
    return nc


def _get_nc():
    if "nc" not in _CACHED:
        _CACHED["nc"] = _build()
    return _CACHED["nc"]


def kernel(hid, ctx, ctx_mask, Wc, Wh, wm, Wo):
    from concourse.bass_utils import run_bass_kernel_spmd

    hid = np.asarray(hid, dtype=np.float32)
    ctx = np.asarray(ctx, dtype=np.float32)
    ctx_mask = np.asarray(ctx_mask, dtype=np.float32)
    Wc = np.asarray(Wc, dtype=np.float32)
    Wh = np.asarray(Wh, dtype=np.float32)
    wm = np.asarray(wm, dtype=np.float32)
    Wo = np.asarray(Wo, dtype=np.float32)

    B = ctx.shape[1]
    assert ctx.shape == (S, B, C) and B == NCORES * BL

    wct = np.ascontiguousarray(Wc.T)
    wht = np.ascontiguousarray(Wh.T).astype(np.float16)
    wot = np.ascontiguousarray(Wo.T)
    wmp = np.ascontiguousarray(wm[0].reshape(KT, 128).T)

    in_maps = []
    for core in range(NCORES):
        b0 = core * BL
        xt = np.ascontiguousarray(ctx[:, b0:b0 + BL, :].transpose(2, 1, 0))
        hidp = np.ascontiguousarray(
            hid[0, b0:b0 + BL, :].T.reshape(KT, 128, BL)
            .transpose(1, 0, 2).reshape(128, KT * BL)).astype(np.float16)
        madd = np.ascontiguousarray(
            np.where(ctx_mask[:, b0:b0 + BL] > 0, 0.0, -1e8).T
        ).astype(np.float32)
        in_maps.append(dict(xt=xt, hidp=hidp, wmp=wmp, maskadd=madd,
                            wct=wct, wht=wht, wot=wot))

    nc = _get_nc()
    res = run_bass_kernel_spmd(nc, in_maps, list(range(NCORES)),
                               trace=TRACE, **TRACE_KWARGS)
    _CACHED["last_result"] = res

    alpha = np.empty((S, B), dtype=np.float32)
    zt = np.empty((B, H), dtype=np.float32)
    for core in range(NCORES):
        b0 = core * BL
        alpha[:, b0:b0 + BL] = res.results[core]["alpha_o"].T
        zt[b0:b0 + BL, :] = res.results[core]["zt_o"]
    return alpha, zt


# revision 43
# speedup vs baseline: 1.0170x; 1.0081x over previous
"""Bahdanau additive attention kernel for Trainium2 (8 NeuronCores, SPMD).

Problem (per reference):
    ctx_  = ctx @ Wc.T          [S,B,MID]   (dominant cost: S*B*C*MID MACs)
    hid_  = hid @ Wh.T          [1,B,MID]
    scores= tanh(ctx_+hid_) @ wm[0]  -> [S,B]
    alpha = softmax_S(mask(scores))
    z     = sum_s alpha * ctx   [B,C]
    z_t   = z @ Wo.T            [B,H]
    returns (alpha, z_t)

Sharding: data-parallel over B (64) across 8 cores -> 8 batches/core.
Device layout (per core); matmul tensors are float32r (fp32 bytes, PE
runs them at bf16 rate for moving dim >= 256 with near-fp32 accuracy):
    xt      [C=1024, b=8, S=1024]   ctx shard transposed (c on partitions)
    hidp    [k=8, p=128, b=8]       hid shard transposed + tiled over H
    wmp     [p=128, j=8]            wm tiled over MID
    maskadd [b=8, S=1024]           0 where mask>0 else -1e8 (additive)
    wct/wht/wot [1024,1024]         Wc.T / Wh.T / Wo.T (replicated)
Outputs: alpha_o [b=8, S=1024], zt_o [b=8, H=1024].

Main loop per batch b: for each 128-wide MID tile j, accumulate
P.T[j] = Wc.T[:,j].T @ xt[:, b, :] over 8 c-tiles (PE, float32r, N=512),
tanh(+hid bias) on ACT into SBUF, then PE-accumulate wm.T @ tanh -> scores.
Softmax on one partition (no max-sub needed: |scores| <= sum|wm| ~ 25).
exp-weights broadcast to 128 partitions (gpsimd), then DVE mul + ACT
accumulate computes zT[c,b] = sum_s xt[c,s]*e[s] per c-tile (softmax
normalization is deferred into a final per-row scale of z_t).
Final z_t = zT.T @ Wo.T on PE.
"""

import sys

import numpy as np

for _p in ("/opt/trn_rl_repo", "/root/.axon_site/_ro/trn_rl_repo"):
    if _p not in sys.path:
        sys.path.insert(0, _p)

S = 1024
C = 1024
H = 1024
MID = 1024
BL = 8  # batches per core
NCORES = 8
KT = 8  # 128-wide tiles over C/H/MID

TRACE = False
TRACE_KWARGS = {}

_CACHED = {}


def _build():
    import concourse.tile as tile
    from concourse import bacc, mybir

    f32 = mybir.dt.float32
    f32r = mybir.dt.float32r
    f16 = mybir.dt.float16
    AF = mybir.ActivationFunctionType

    nc = bacc.Bacc("TRN2", target_bir_lowering=False, debug=False,
                   num_devices=NCORES)

    xt = nc.dram_tensor("xt", [C, BL, S], f32r, kind="ExternalInput").ap()
    hidp = nc.dram_tensor("hidp", [128, KT * BL], f16, kind="ExternalInput").ap()
    wmp = nc.dram_tensor("wmp", [128, KT], f32r, kind="ExternalInput").ap()
    maskadd = nc.dram_tensor("maskadd", [BL, S], f32, kind="ExternalInput").ap()
    wct = nc.dram_tensor("wct", [C, MID], f32r, kind="ExternalInput").ap()
    wht = nc.dram_tensor("wht", [H, MID], f16, kind="ExternalInput").ap()
    wot = nc.dram_tensor("wot", [C, H], f32r, kind="ExternalInput").ap()
    alpha_o = nc.dram_tensor("alpha_o", [BL, S], f32, kind="ExternalOutput").ap()
    zt_o = nc.dram_tensor("zt_o", [BL, H], f32, kind="ExternalOutput").ap()

    with tile.TileContext(nc) as tc:
        consts = tc.alloc_tile_pool(name="consts", bufs=1)
        # resident weights / small constants
        wct_sb = [consts.tile([128, MID], f32r, name=f"wct{k}") for k in range(KT)]
        wot_sb = [consts.tile([128, H], f32r, name=f"wot{k}") for k in range(KT)]
        hidp_sb = consts.tile([128, KT * BL], f16, name="hidp_sb")
        hid_sb = [consts.tile([128, BL], f32, name=f"hid{j}") for j in range(KT)]
        wm_sb = consts.tile([128, KT], f32r, name="wm_sb")
        mask_sb = consts.tile([1, BL * S], f32, name="mask_sb")
        zT_sb = [consts.tile([128, BL], f32, name=f"zT{k}") for k in range(KT)]

        nc.sync.dma_start(out=hidp_sb, in_=hidp)

        # ---- hid projection: hid_T[m, b] = sum_h WhT[h, m] * hidT[h, b] ----
        whtp = tc.alloc_tile_pool(name="whtp", bufs=3)
        hps = tc.alloc_tile_pool(name="hps", bufs=1, space="PSUM")
        hid_ps = [hps.tile([128, BL], f32, name=f"hidps{j}", tag=f"hidps{j}")
                  for j in range(KT)]
        for k in range(KT):
            whtt = whtp.tile([128, MID], f16, name=f"whtt{k}", tag="wht")
            nc.sync.dma_start(out=whtt, in_=wht[k * 128:(k + 1) * 128, :])
            for j in range(KT):
                nc.tensor.matmul(
                    hid_ps[j], lhsT=whtt[:, j * 128:(j + 1) * 128],
                    rhs=hidp_sb[:, k * BL:(k + 1) * BL],
                    start=(k == 0), stop=(k == KT - 1))
        for j in range(KT):
            nc.vector.tensor_copy(hid_sb[j], hid_ps[j])
        hps.release()
        whtp.release()

        for k in range(KT):
            nc.sync.dma_start(out=wct_sb[k][:, 0:512],
                              in_=wct[k * 128:(k + 1) * 128, 0:512])
        nc.sync.dma_start(out=wm_sb, in_=wmp)
        nc.sync.dma_start(out=mask_sb,
                          in_=maskadd.rearrange("b s -> (b s)").unsqueeze(0))

        # ---- main per-batch loop ----
        xtp = tc.alloc_tile_pool(name="xtp", bufs=16)
        tanp = tc.alloc_tile_pool(name="tanp", bufs=3)
        scrp = tc.alloc_tile_pool(name="scrp", bufs=2)
        abcp = tc.alloc_tile_pool(name="abcp", bufs=2)
        smallp = tc.alloc_tile_pool(name="smallp", bufs=2)
        pp = tc.alloc_tile_pool(name="pp", bufs=2, space="PSUM")
        scp = tc.alloc_tile_pool(name="scp", bufs=4, space="PSUM")

        sume_row = consts.tile([1, BL], f32, name="sume_row")
        rcp_col = consts.tile([BL, 1], f32, name="rcp_col")
        rcp_row = consts.tile([1, BL], f32, name="rcp_row")

        pending = None  # (b_prev, xtt_prev, abc_prev): phase-3 work deferred
        for b in range(BL):
            xtt = [xtp.tile([128, S], f32r, name=f"xt{b}_{k}", tag="xt")
                   for k in range(KT)]
            for k in range(KT):
                nc.sync.dma_start(out=xtt[k],
                                  in_=xt[k * 128:(k + 1) * 128, b, :])
            if b == 0:
                for k in range(KT):
                    nc.sync.dma_start(out=wct_sb[k][:, 512:1024],
                                      in_=wct[k * 128:(k + 1) * 128, 512:1024])

            sc_a = scp.tile([1, 512], f32, name=f"sca{b}", tag="sc")
            sc_b = scp.tile([1, 512], f32, name=f"scb{b}", tag="sc")
            for j in range(KT):
                pt = pp.tile([128, S], f32, name=f"pt{b}_{j}", tag="pp")
                for k in range(KT):
                    for h in range(2):
                        nc.tensor.matmul(
                            pt[:, h * 512:(h + 1) * 512],
                            lhsT=wct_sb[k][:, j * 128:(j + 1) * 128],
                            rhs=xtt[k][:, h * 512:(h + 1) * 512],
                            start=(k == 0), stop=(k == KT - 1))
                tt = tanp.tile([128, S], f32r, name=f"tt{b}_{j}", tag="T")
                nc.scalar.activation(out=tt, in_=pt, func=AF.Tanh,
                                     bias=hid_sb[j][:, b:b + 1], scale=1.0)
                nc.tensor.matmul(sc_a, lhsT=wm_sb[:, j:j + 1],
                                 rhs=tt[:, 0:512],
                                 start=(j == 0), stop=(j == KT - 1))
                nc.tensor.matmul(sc_b, lhsT=wm_sb[:, j:j + 1],
                                 rhs=tt[:, 512:1024],
                                 start=(j == 0), stop=(j == KT - 1))
                if pending is not None:
                    pb, pxtt, pabc = pending
                    prod = scrp.tile([128, S], f32, name=f"prod{pb}_{j}",
                                     tag="prod")
                    nc.vector.tensor_mul(prod, pxtt[j].bitcast(f32), pabc)
                    nc.scalar.activation(out=prod, in_=prod, func=AF.Copy,
                                         accum_out=zT_sb[j][:, pb:pb + 1])

            # scores (+ additive mask) on partition 0
            scores = smallp.tile([1, S], f32, name=f"scores{b}", tag="scores")
            nc.vector.tensor_add(scores[:, 0:512], sc_a,
                                 mask_sb[:, b * S:b * S + 512])
            nc.vector.tensor_add(scores[:, 512:1024], sc_b,
                                 mask_sb[:, b * S + 512:b * S + 1024])

            # softmax over S (no max subtraction; |scores| is bounded ~25).
            # Normalization is deferred: broadcast raw exp, scale z_t at end.
            nc.scalar.activation(out=scores, in_=scores, func=AF.Exp,
                                 accum_out=sume_row[:, b:b + 1])

            # weighted (unnormalized) context zT[c,b] = sum_s xt[c,s]*e[s]
            # is deferred into the next batch's j-loop (ACT/DVE interleave)
            abc = abcp.tile([128, S], f32, name=f"abc{b}", tag="abc")
            nc.gpsimd.partition_broadcast(abc, scores)
            pending = (b, xtt, abc)

            # normalized alpha output (off the critical chain)
            rcp = smallp.tile([1, 1], f32, name=f"rcp{b}", tag="rcp")
            nc.vector.reciprocal(rcp, sume_row[:, b:b + 1])
            nc.vector.tensor_scalar_mul(scores, scores, rcp)
            nc.sync.dma_start(out=alpha_o[b:b + 1, :], in_=scores)

        # drain the last batch's deferred phase-3
        pb, pxtt, pabc = pending
        for k in range(KT):
            prod = scrp.tile([128, S], f32, name=f"prodL{k}", tag="prod")
            nc.vector.tensor_mul(prod, pxtt[k].bitcast(f32), pabc)
            nc.scalar.activation(out=prod, in_=prod, func=AF.Copy,
                                 accum_out=zT_sb[k][:, pb:pb + 1])

        # ---- z_t = zT.T @ WoT ----
        scp.release()
        pp.release()
        for k in range(KT):
            nc.sync.dma_start(out=wot_sb[k], in_=wot[k * 128:(k + 1) * 128, :])
        zTr = [consts.tile([128, BL], f32r, name=f"zTr{k}") for k in range(KT)]
        ztp = tc.alloc_tile_pool(name="ztp", bufs=1, space="PSUM")
        zt_ps = [ztp.tile([BL, 512], f32, name=f"ztps{h}", tag=f"ztps{h}")
                 for h in range(2)]
        for k in range(KT):
            nc.vector.tensor_copy(zTr[k], zT_sb[k])
            for h in range(2):
                nc.tensor.matmul(zt_ps[h], lhsT=zTr[k],
                                 rhs=wot_sb[k][:, h * 512:(h + 1) * 512],
                                 start=(k == 0), stop=(k == KT - 1))
        nc.vector.reciprocal(rcp_row, sume_row)
        nc.sync.dma_start(out=rcp_col, in_=rcp_row)
        zt_sb = smallp.tile([BL, H], f32, name="zt_sb", tag="ztsb")
        for h in range(2):
            nc.vector.tensor_scalar_mul(zt_sb[:, h * 512:(h + 1) * 512],
                                        zt_ps[h], rcp_col)
        nc.sync.dma_start(out=zt_o, in_=zt_sb)

        # release in LIFO order per space (SBUF / PSUM stacks)
        ztp.release()
        smallp.release()
        abcp.release()
        scrp.release()
        tanp.release()
        xtp.release()
        consts.release()

    nc.compile()
    return nc


def _get_nc():
    if "nc" not in _CACHED:
        _CACHED["nc"] = _build()
    return _CACHED["nc"]


def kernel(hid, ctx, ctx_mask, Wc, Wh, wm, Wo):
    from concourse.bass_utils import run_bass_kernel_spmd

    hid = np.asarray(hid, dtype=np.float32)
    ctx = np.asarray(ctx, dtype=np.float32)
    ctx_mask = np.asarray(ctx_mask, dtype=np.float32)
    Wc = np.asarray(Wc, dtype=np.float32)
    Wh = np.asarray(Wh, dtype=np.float32)
    wm = np.asarray(wm, dtype=np.float32)
    Wo = np.asarray(Wo, dtype=np.float32)

    B = ctx.shape[1]
    assert ctx.shape == (S, B, C) and B == NCORES * BL

    wct = np.ascontiguousarray(Wc.T)
    wht = np.ascontiguousarray(Wh.T).astype(np.float16)
    wot = np.ascontiguousarray(Wo.T)
    wmp = np.ascontiguousarray(wm[0].reshape(KT, 128).T)

    in_maps = []
    for core in range(NCORES):
        b0 = core * BL
        xt = np.ascontiguousarray(ctx[:, b0:b0 + BL, :].transpose(2, 1, 0))
        hidp = np.ascontiguousarray(
            hid[0, b0:b0 + BL, :].T.reshape(KT, 128, BL)
            .transpose(1, 0, 2).reshape(128, KT * BL)).astype(np.float16)
        madd = np.ascontiguousarray(
            np.where(ctx_mask[:, b0:b0 + BL] > 0, 0.0, -1e8).T
        ).astype(np.float32)
        in_maps.append(dict(xt=xt, hidp=hidp, wmp=wmp, maskadd=madd,
                            wct=wct, wht=wht, wot=wot))

    nc = _get_nc()
    res = run_bass_kernel_spmd(nc, in_maps, list(range(NCORES)),
                               trace=TRACE, **TRACE_KWARGS)
    _CACHED["last_result"] = res

    alpha = np.empty((S, B), dtype=np.float32)
    zt = np.empty((B, H), dtype=np.float32)
    for core in range(NCORES):
        b0 = core * BL
        alpha[:, b0:b0 + BL] = res.results[core]["alpha_o"].T
        zt[b0:b0 + BL, :] = res.results[core]["zt_o"]
    return alpha, zt
